# revision 37
# baseline (speedup 1.0000x reference)
"""Trainium2 Bass kernel: attention layer with RoPE + gated adapter cross-attention.

Problem: B=2, S=2048, D=2048, H=16 heads (HD=128), adapter_len L=10.

  xq/xk/xv = x @ wq/wk/wv   (per-head reshape)
  xq, xk = rope(xq), rope(xk)
  out  = softmax(xq xk^T * scale + causal_mask) @ xv
  out += gate_h * softmax(xq ak^T * scale) @ av     (ak/av = adapter @ wk/wv)
  y    = out @ wo

Sharding (8 NeuronCores): 2 batch shards x 4 head-groups of 4 heads.
Each core computes attention for its (batch, 4 heads) and the partial
output projection with its 512 rows of wo; the host sums 4 partials per
batch element.  No on-device collectives.

Device layouts (per core):
  xT     [D, S]    bf16  x[b] transposed (feature-major)
  wq/wk  [D, 512]  bf16  column slice, RoPE-deinterleave column permutation
  wv     [D, 512]  bf16  column slice (no permutation)
  wo     [512, D]  bf16  row slice (no permutation)
  csq2   [128, S]  bf16  rows 0:64 cos.T*scale, 64:128 sin.T*scale
  csq2s  [128, S]  bf16  rows 0:64 sin.T*scale, 64:128 cos.T*scale
  csk2   [128, S]  bf16  [cos.T; sin.T]  (for k, unscaled)
  csk2s  [128, S]  bf16  [sin.T; cos.T]
  akT    [128, 40] bf16  adapter-K transposed per head (host-computed)
  avg    [10, 512] bf16  adapter-V, pre-scaled by per-head gate (host)
  y      [S, D]    f32   partial output (ExternalOutput)

The RoPE trick: permuting wq/wk columns so each head's features are
[even0..even63, odd0..odd63] makes the rotation act on partition halves.
With cos/sin packed as [c;s] and [s;c] 128-partition tiles:
  A = proj * [c;s]   -> [v0*c ; v1*s]   (one full-width DVE mult)
  B = proj * [s;c]   -> [v0*s ; v1*c]
  top = A[0:64] - A[64:128];  bot = B[0:64] + B[64:128]
All bf16 (2x DVE rate); the f32 PSUM result is downcast once on ACT.

Softmax: scores computed transposed ([k, q]); constant-shift exp(s - 8)
(softmax-invariant); denominator via ones-vector matmul accumulated
alongside PV.  Normalizers 1/sum computed as Exp(-Ln(sum)) on the ACT
engine ([1,512] DVE reciprocal measured ~4us each -- ACT pair is ~0.7us),
then partition-broadcast on GpSimd.  The adapter gate is folded into the
host-computed adapter-V, so no gate multiply on device.

Schedule: fully streamed per 512-query J-block --
  QK-proj+RoPE(J) -> V-proj(J) -> attention(h=0..3, J) -> wo(J-1)
so the PE never waits on a phase boundary and the wo matmuls trail one
block behind the epilogues they consume.
"""

import numpy as np
import ml_dtypes

B, S, D, H, HD, L = 2, 2048, 2048, 16, 128, 10
NCORES = 8
NG = 4            # head-group shards
NH = H // NG      # heads per core
DH = NH * HD      # 512: per-core projection width
QT = 512          # query chunk (free dim of most matmuls)
NJ = S // QT      # 4
KT = 128          # key tile
DKT = 128         # contraction tile
NDK = D // DKT    # 16
NST = S // 128    # 16 s-tiles
SCALE = 1.0 / float(np.sqrt(HD))

_BF16 = ml_dtypes.bfloat16
_NC_CACHE = {}


def _build_nc():
    """Build + compile the per-core Bacc graph (same graph on all cores)."""
    from contextlib import ExitStack

    import concourse.tile as tile
    from concourse import bacc, mybir

    f32, bf16 = mybir.dt.float32, mybir.dt.bfloat16
    AF = mybir.ActivationFunctionType
    OP = mybir.AluOpType

    nc = bacc.Bacc("TRN2", target_bir_lowering=False, debug=False,
                   num_devices=NCORES)
    xT = nc.dram_tensor("xT", [D, S], bf16, kind="ExternalInput").ap()
    wq = nc.dram_tensor("wq", [D, DH], bf16, kind="ExternalInput").ap()
    wk = nc.dram_tensor("wk", [D, DH], bf16, kind="ExternalInput").ap()
    wv = nc.dram_tensor("wv", [D, DH], bf16, kind="ExternalInput").ap()
    wo = nc.dram_tensor("wo", [DH, D], bf16, kind="ExternalInput").ap()
    csq2 = nc.dram_tensor("csq2", [128, S], bf16, kind="ExternalInput").ap()
    csq2s = nc.dram_tensor("csq2s", [128, S], bf16, kind="ExternalInput").ap()
    csk2 = nc.dram_tensor("csk2", [128, S], bf16, kind="ExternalInput").ap()
    csk2s = nc.dram_tensor("csk2s", [128, S], bf16, kind="ExternalInput").ap()
    akT = nc.dram_tensor("akT", [128, NH * L], bf16, kind="ExternalInput").ap()
    avg = nc.dram_tensor("avg", [L, DH], bf16, kind="ExternalInput").ap()
    dmask = nc.dram_tensor("dmask", [128, 4 * QT], bf16,
                           kind="ExternalInput").ap()
    y = nc.dram_tensor("y", [S, D], bf16, kind="ExternalOutput").ap()

    with tile.TileContext(nc) as tc:
        with ExitStack() as ctx:
            pers = ctx.enter_context(tc.tile_pool(name="pers", bufs=1))
            ps = ctx.enter_context(
                tc.tile_pool(name="ps", space="PSUM", bufs=4))
            sb = ctx.enter_context(tc.tile_pool(name="sb", bufs=4))

            def ptile(shape, dt, nm):
                return pers.tile(shape, dt, name=nm, tag=nm)

            wq_t = [ptile([128, DH], bf16, f"twq{dk}") for dk in range(NDK)]
            wk_t = [ptile([128, DH], bf16, f"twk{dk}") for dk in range(NDK)]
            wv_t = [ptile([128, DH], bf16, f"twv{dk}") for dk in range(NDK)]
            wo_t = [ptile([128, D], bf16, f"two{f}") for f in range(NH)]
            csq2_t = ptile([128, S], bf16, "tcsq2")
            csq2s_t = ptile([128, S], bf16, "tcsq2s")
            csk2_t = ptile([128, S], bf16, "tcsk2")
            csk2s_t = ptile([128, S], bf16, "tcsk2s")
            akT_t = ptile([128, NH * L], bf16, "takT")
            avg_t = ptile([L, DH], bf16, "tavg")
            ones_t = ptile([128, 128], bf16, "tones")
            m8_t = ptile([128, 1], f32, "tm8")
            dm_t = ptile([128, 4 * QT], bf16, "tdm")
            kT_t = [ptile([128, S], bf16, f"tkT{h}") for h in range(NH)]
            v_t = [ptile([128, DH], bf16, f"tv{si}") for si in range(NST)]

            xt_all = {}

            def load_x(J):
                """Emit x DMAs for block J; J=0 interleaves wq/wk so the
                first accumulation group's operands land ASAP (in-order
                DMA queues: emission order = arrival order)."""
                jsl = slice(J * QT, (J + 1) * QT)
                xt = []
                for dk in range(NDK):
                    t = sb.tile([128, QT], bf16, tag="x", bufs=22,
                                name=f"x{J}_{dk}")
                    nc.sync.dma_start(
                        t[:], xT[dk * 128:(dk + 1) * 128, jsl])
                    xt.append(t)
                    if J == 0:
                        # block 0 computes V first: pair x with wv so the
                        # PE can start after ~2 tiles instead of 6MB
                        nc.sync.dma_start(
                            wv_t[dk][:], wv[dk * 128:(dk + 1) * 128, :])
                xt_all[J] = xt

            # ---- initial DMA emission (consumption order)
            load_x(0)
            for dk in range(NDK):
                nc.sync.dma_start(wq_t[dk][:], wq[dk * 128:(dk + 1) * 128, :])
                nc.sync.dma_start(wk_t[dk][:], wk[dk * 128:(dk + 1) * 128, :])
            for t, src in ((csq2_t, csq2), (csq2s_t, csq2s),
                           (csk2_t, csk2), (csk2s_t, csk2s)):
                nc.sync.dma_start(t[:], src[:, :])
            nc.sync.dma_start(dm_t[:], dmask[:, :])
            nc.sync.dma_start(akT_t[:], akT[:, :])
            nc.sync.dma_start(avg_t[:], avg[:, :])
            nc.gpsimd.memset(ones_t[:], 1.0)
            nc.gpsimd.memset(m8_t[:], -8.0)
            load_x(1)
            for f in range(NH):
                nc.sync.dma_start(wo_t[f][:], wo[f * 128:(f + 1) * 128, :])

            ao_all = {}   # (h, J) -> [128, QT] bf16 unnormalized-combined

            def emit_wo(J):
                """Output projection for s-tiles of block J (trails one
                block behind attention so epilogues are long done)."""
                fine = (J == NJ - 1)  # last block: small stores spread the
                for sv in range(4):   # tail across more DMA queues
                    si = 4 * J + sv
                    ssl = slice(si * 128, (si + 1) * 128)
                    lsl = slice(sv * 128, (sv + 1) * 128)
                    for half in range(2):
                        ysb = sb.tile([128, D // 2], bf16, tag="y", bufs=3,
                                      name=f"y{si}_{half}")
                        for n2 in range(2):
                            n = 2 * half + n2
                            nsl = slice(n * QT, (n + 1) * QT)
                            yp = ps.tile([128, QT], f32, tag="s", bufs=4,
                                         name=f"yp{si}_{n}")
                            for f in range(NH):
                                nc.tensor.matmul(
                                    yp[:], ao_all[(f, J)][:, lsl],
                                    wo_t[f][:, nsl],
                                    start=(f == 0), stop=(f == NH - 1))
                            nc.scalar.copy(
                                ysb[:, n2 * QT:(n2 + 1) * QT], yp[:])
                            if fine:
                                nc.sync.dma_start(
                                    y[ssl, nsl],
                                    ysb[:, n2 * QT:(n2 + 1) * QT])
                        if not fine:
                            nc.sync.dma_start(
                                y[ssl, half * (D // 2):(half + 1) * (D // 2)],
                                ysb[:])

            for J in range(NJ):
                if 1 <= J and J + 1 < NJ:
                    load_x(J + 1)
                jsl = slice(J * QT, (J + 1) * QT)
                xt = xt_all[J]

                def emit_v(Jv):
                    xv = xt_all[Jv]
                    for sv in range(4):
                        si = 4 * Jv + sv
                        vp = ps.tile([128, DH], f32, tag="s", bufs=4,
                                     name=f"vp{si}")
                        for dk in range(NDK):
                            nc.tensor.matmul(
                                vp[:], xv[dk][:, sv * 128:(sv + 1) * 128],
                                wv_t[dk][:], start=(dk == 0),
                                stop=(dk == NDK - 1))
                        nc.scalar.copy(v_t[si][:], vp[:])

                # block 0 races the initial DMA: V-proj needs only x+wv
                # (4MB), so run it first while wq/wk are still streaming
                if J == 0:
                    emit_v(0)

                # ---- QK projections + RoPE for this block, per head
                qT_J = {}
                for h in range(NH):
                    hsl = slice(h * 128, (h + 1) * 128)
                    for w_t, cs2, cs2s, pfx in (
                            (wq_t, csq2_t, csq2s_t, "q"),
                            (wk_t, csk2_t, csk2s_t, "k")):
                        pp = ps.tile([128, QT], f32, tag="s", bufs=4,
                                     name=f"pp{pfx}{J}_{h}")
                        for dk in range(NDK):
                            nc.tensor.matmul(
                                pp[:], w_t[dk][:, hsl], xt[dk][:],
                                start=(dk == 0), stop=(dk == NDK - 1))
                        psb = sb.tile([128, QT], bf16, tag="psb", bufs=4,
                                      name=f"pb{pfx}{J}_{h}")
                        nc.scalar.copy(psb[:], pp[:])
                        if pfx == "q":
                            dst = sb.tile([128, QT], bf16, tag="qT", bufs=6,
                                          name=f"qT{J}_{h}")
                            qT_J[h] = dst
                            d_top, d_bot = dst[0:64, :], dst[64:128, :]
                        else:
                            d_top = kT_t[h][0:64, jsl]
                            d_bot = kT_t[h][64:128, jsl]
                        # all-bf16 products; DVE two-SBUF-input ops need
                        # matching base partitions, so pair 0:64 with 0:64
                        # and 64:128 with 64:128 (cs2=[c;s], cs2s=[s;c])
                        tt = []
                        for i, (vsl, cst) in enumerate((
                                (slice(0, 64), cs2), (slice(64, 128), cs2),
                                (slice(0, 64), cs2s),
                                (slice(64, 128), cs2s))):
                            t = sb.tile([64, QT], bf16, tag="rt", bufs=8,
                                        name=f"t{i}{pfx}{J}_{h}")
                            nc.vector.tensor_tensor(
                                t[:], psb[vsl, :], cst[vsl, jsl], op=OP.mult)
                            tt.append(t)
                        nc.vector.tensor_tensor(
                            d_top, tt[0][:], tt[1][:], op=OP.subtract)
                        nc.vector.tensor_tensor(
                            d_bot, tt[2][:], tt[3][:], op=OP.add)

                # ---- V projection for this block's 4 s-tiles
                if J > 0:
                    emit_v(J)

                # ---- attention for each head at this block
                for h in range(NH):
                    hsl = slice(h * 128, (h + 1) * 128)
                    qs = qT_J[h][:, :]
                    nki = 4 * J + 4
                    ops_ = ps.tile([128, QT], f32, tag="o", bufs=2,
                                   name=f"o{h}_{J}")
                    sums = ps.tile([128, QT], f32, tag="sum", bufs=2,
                                   name=f"sm{h}_{J}")
                    # adapter scores issue before the ki loop; their exp
                    # latency hides behind the main attention matmuls
                    ap_ = ps.tile([L, QT], f32, tag="s", bufs=4,
                                  name=f"ap{h}_{J}")
                    nc.tensor.matmul(ap_[:], akT_t[:, h * L:(h + 1) * L],
                                     qs, start=True, stop=True)
                    pa = sb.tile([L, QT], bf16, tag="pa", bufs=2,
                                 name=f"pa{h}_{J}")
                    nc.scalar.activation(pa[:], ap_[:], AF.Exp,
                                         bias=m8_t[0:L, :])
                    # software-pipelined ki loop (depth 2): the scores
                    # matmul of ki+1..ki+2 issues on PE before the PV of
                    # ki, hiding the PE->ACT(exp)->PE round trip.
                    #
                    # Denominator: M=128 ones matmuls (the result arrives
                    # already partition-broadcast, so no GpSimd step), one
                    # per PAIR of prob tiles pre-added on the idle DVE.
                    # This halves the sums PE cost and balances the inner
                    # loop: PE ~540ns/ki vs ACT exp ~547ns/ki (dropping
                    # sums entirely makes attention ACT-bound and stalls
                    # the in-order PE queue -- measured slower).
                    pend = []
                    half = []
                    n_sums = (nki + 1) // 2
                    sums_emitted = 0

                    def emit_sums(a, b):
                        nonlocal sums_emitted
                        ka, ta, qa = a
                        if b is not None:
                            kb, tb, qb = b
                            # in-place pair add on b's visible range; the
                            # [qa:qb) region keeps a's values alone
                            nc.vector.tensor_tensor(
                                ta[:, qb:], ta[:, qb:], tb[:, qb:],
                                op=OP.add)
                        nc.tensor.matmul(
                            sums[:, qa:], ones_t[:, :], ta[:, qa:],
                            start=(sums_emitted == 0),
                            stop=(sums_emitted == n_sums - 1),
                            skip_group_check=True)
                        sums_emitted += 1

                    def flush_one():
                        ki, pt_t, q0 = pend.pop(0)
                        nc.tensor.matmul(
                            ops_[:, q0:], v_t[ki][:, hsl], pt_t[:, q0:],
                            start=(ki == 0), stop=(ki == nki - 1),
                            skip_group_check=True)
                        half.append((ki, pt_t, q0))
                        if len(half) == 2:
                            emit_sums(half[0], half[1])
                            half.clear()

                    for ki in range(nki):
                        di = ki - 4 * J
                        q0 = di * 128 if di >= 0 else 0
                        sp = ps.tile([128, QT], f32, tag="s", bufs=4,
                                     name=f"sp{h}_{J}_{ki}")
                        nc.tensor.matmul(
                            sp[:, q0:], kT_t[h][:, ki * KT:(ki + 1) * KT],
                            qs[:, q0:], start=True, stop=True)
                        pt = sb.tile([128, QT], bf16, tag="pt", bufs=6,
                                     name=f"pt{h}_{J}_{ki}")
                        # exp(s - 8): softmax-invariant shift guards
                        # f32 overflow for any plausible score scale
                        nc.scalar.activation(pt[:, q0:], sp[:, q0:],
                                             AF.Exp, bias=m8_t[:, :])
                        if di >= 0:
                            ptm = sb.tile([128, QT], bf16, tag="ptm",
                                          bufs=4, name=f"pm{h}_{J}_{ki}")
                            nc.vector.tensor_tensor(
                                ptm[:, q0:], pt[:, q0:],
                                dm_t[:, di * QT + q0:(di + 1) * QT],
                                op=OP.mult)
                            pt = ptm
                        pend.append((ki, pt, q0))
                        # depth 2 early (ap_ still holds the 4th "s" PSUM
                        # buf), depth 3 once it frees after the pa exp
                        if len(pend) > (2 if ki < 4 else 3):
                            flush_one()
                    while pend:
                        flush_one()
                    if half:
                        emit_sums(half[0], None)
                        half.clear()
                    # adapter PV + sums (pa ready by now)
                    asums = ps.tile([128, QT], f32, tag="sum", bufs=2,
                                    name=f"as{h}_{J}")
                    nc.tensor.matmul(asums[:], ones_t[0:L, :], pa[:],
                                     start=True, stop=True)
                    avp_ = ps.tile([128, QT], f32, tag="s", bufs=4,
                                   name=f"av{h}_{J}")
                    nc.tensor.matmul(avp_[:], avg_t[:, hsl], pa[:],
                                     start=True, stop=True)
                    # epilogue: 1/sum via the fast custom-DVE reciprocal
                    # (~5x the plain DVE reciprocal; sums are positive and
                    # well inside range; DVE cost scales with free size
                    # only, so the [128,QT] broadcast shape is free).  ACT
                    # Ln/Exp would be fast per-op but each function switch
                    # costs a ~1.3us table reload.
                    # adapter term first: frees avp_'s "s" PSUM buf ASAP
                    # (it otherwise gates the next block's proj allocs)
                    rab = sb.tile([128, QT], f32, tag="ep2", bufs=2,
                                  name=f"rc{h}_{J}")
                    nc.vector.reciprocal_approx_fast(out=rab[:],
                                                     in_=asums[:])
                    t_a = sb.tile([128, QT], f32, tag="ep3", bufs=2,
                                  name=f"ta{h}_{J}")
                    nc.vector.tensor_tensor(t_a[:], avp_[:], rab[:],
                                            op=OP.mult)
                    rb = sb.tile([128, QT], f32, tag="ep2", bufs=2,
                                 name=f"rb{h}_{J}")
                    nc.vector.reciprocal_approx_fast(out=rb[:], in_=sums[:])
                    t_o = sb.tile([128, QT], f32, tag="ep3", bufs=2,
                                  name=f"to{h}_{J}")
                    nc.vector.tensor_tensor(t_o[:], ops_[:], rb[:],
                                            op=OP.mult)
                    ao = sb.tile([128, QT], bf16, tag="ao", bufs=8,
                                 name=f"ao{h}_{J}")
                    ao_all[(h, J)] = ao
                    nc.vector.tensor_tensor(ao[:], t_o[:], t_a[:], op=OP.add)

                # ---- output projection trails one block behind
                if J >= 1:
                    emit_wo(J - 1)
            emit_wo(NJ - 1)
    nc.compile()
    return nc


def get_nc():
    if "nc" not in _NC_CACHE:
        _NC_CACHE["nc"] = _build_nc()
    return _NC_CACHE["nc"]


# ---------------------------------------------------------------- host side

def _rope_perm():
    """Column permutation making each head's features [evens..., odds...]."""
    blk = np.concatenate([np.arange(0, 128, 2), np.arange(1, 128, 2)])
    return np.concatenate([h * 128 + blk for h in range(NH)])


def _diag_masks():
    """dmask[k_local, di*QT + q_local] = 1 if (di*128 + k_local) <= q_local."""
    out = np.zeros((128, 4 * QT), dtype=np.float32)
    kl = np.arange(128)[:, None]
    ql = np.arange(QT)[None, :]
    for di in range(4):
        out[:, di * QT:(di + 1) * QT] = (di * 128 + kl) <= ql
    return out


def make_core_inputs(inputs, b, hg):
    """Build the in_map for core (b, hg). All arrays C-contiguous."""
    x = np.asarray(inputs["x"], dtype=np.float32)
    wq = np.asarray(inputs["wq"], dtype=np.float32)
    wk = np.asarray(inputs["wk"], dtype=np.float32)
    wv = np.asarray(inputs["wv"], dtype=np.float32)
    wo = np.asarray(inputs["wo"], dtype=np.float32)
    adapter = np.asarray(inputs["adapter"], dtype=np.float32)
    gate = np.asarray(inputs["gate"], dtype=np.float32)
    cos = np.asarray(inputs["freqs_cos"], dtype=np.float32)
    sin = np.asarray(inputs["freqs_sin"], dtype=np.float32)

    cols = slice(hg * DH, (hg + 1) * DH)
    perm = _rope_perm()
    bf = _BF16
    wkp = np.ascontiguousarray(wk[:, cols][:, perm])
    wvs = np.ascontiguousarray(wv[:, cols])
    # adapter K/V on host (tiny): ak in the same permuted feature basis as
    # q; av pre-scaled by the per-head gate so no gate multiply on device
    ak = adapter[0] @ wkp                      # [L, DH] permuted features
    av = adapter[0] @ wvs                      # [L, DH]
    gv = gate[0, hg * NH:(hg + 1) * NH, 0, 0]  # [NH]
    avg_ = (av.reshape(L, NH, HD) * gv[None, :, None]).reshape(L, DH)
    akT_ = np.concatenate(
        [ak[:, h * HD:(h + 1) * HD].T for h in range(NH)], axis=1)  # [128,40]
    cT, sT = cos.T, sin.T                      # [64, S]
    m = {
        "xT": np.ascontiguousarray(x[b].T).astype(bf),
        "wq": np.ascontiguousarray(wq[:, cols][:, perm]).astype(bf),
        "wk": wkp.astype(bf),
        "wv": wvs.astype(bf),
        "wo": np.ascontiguousarray(wo[cols, :]).astype(bf),
        "csq2": np.ascontiguousarray(
            np.concatenate([cT * SCALE, sT * SCALE], axis=0)).astype(bf),
        "csq2s": np.ascontiguousarray(
            np.concatenate([sT * SCALE, cT * SCALE], axis=0)).astype(bf),
        "csk2": np.ascontiguousarray(
            np.concatenate([cT, sT], axis=0)).astype(bf),
        "csk2s": np.ascontiguousarray(
            np.concatenate([sT, cT], axis=0)).astype(bf),
        "akT": np.ascontiguousarray(akT_).astype(bf),
        "avg": np.ascontiguousarray(avg_).astype(bf),
        "dmask": _diag_masks().astype(bf),
    }
    return m


def _mask_is_causal(mask):
    """True when mask[0,0] is the standard additive causal mask."""
    mk = np.asarray(mask)[0, 0]
    iu = np.triu_indices(S, k=1)
    il = np.tril_indices(S, k=0)
    return bool(np.all(mk[il] == 0.0) and np.all(mk[iu] < -1e8))


def _host_fallback(inputs):
    """Pure-numpy reference (used only if the mask is not causal)."""
    x = np.asarray(inputs["x"], dtype=np.float32)
    wq = np.asarray(inputs["wq"], dtype=np.float32)
    wk = np.asarray(inputs["wk"], dtype=np.float32)
    wv = np.asarray(inputs["wv"], dtype=np.float32)
    wo = np.asarray(inputs["wo"], dtype=np.float32)
    adapter = np.asarray(inputs["adapter"], dtype=np.float32)
    gate = np.asarray(inputs["gate"], dtype=np.float32)
    cos = np.asarray(inputs["freqs_cos"], dtype=np.float32)
    sin = np.asarray(inputs["freqs_sin"], dtype=np.float32)
    mask = np.asarray(inputs["mask"], dtype=np.float32)

    def rope(v):
        vv = v.reshape(*v.shape[:-1], HD // 2, 2)
        v0, v1 = vv[..., 0], vv[..., 1]
        c = cos[None, :, None, :]
        s = sin[None, :, None, :]
        out = np.stack([v0 * c - v1 * s, v0 * s + v1 * c], axis=-1)
        return out.reshape(v.shape)

    xq = rope((x @ wq).reshape(B, S, H, HD))
    xk = rope((x @ wk).reshape(B, S, H, HD))
    xv = (x @ wv).reshape(B, S, H, HD)
    scores = np.einsum("bqhd,bkhd->bhqk", xq, xk) * SCALE + mask
    scores -= scores.max(axis=-1, keepdims=True)
    p = np.exp(scores)
    p /= p.sum(axis=-1, keepdims=True)
    out = np.einsum("bhqk,bkhd->bqhd", p, xv)
    ak = (adapter[0] @ wk).reshape(L, H, HD)
    av = (adapter[0] @ wv).reshape(L, H, HD)
    asc = np.einsum("bqhd,khd->bhqk", xq, ak) * SCALE
    asc -= asc.max(axis=-1, keepdims=True)
    pa = np.exp(asc)
    pa /= pa.sum(axis=-1, keepdims=True)
    pa = gate * pa
    out = out + np.einsum("bhqk,khd->bqhd", pa, av)
    return (out.reshape(B, S, D) @ wo).astype(np.float32)


def _device_available():
    """Check the axon tunnel is reachable without claiming a device (a jax
    probe subprocess would grab a terminal session and could contend with
    the real run).  When no tunnel env is present, assume native devices."""
    import os
    import socket

    if not os.environ.get("TRN_TERMINAL_POOL_IPS"):
        import glob

        return bool(glob.glob("/dev/neuron*"))  # native path
    for port in (8082, 8083, 8087):
        s = socket.socket()
        s.settimeout(5)
        try:
            s.connect(("127.0.0.1", port))
            return True
        except OSError:
            continue
        finally:
            s.close()
    return False


def kernel(**inputs) -> np.ndarray:
    if not _mask_is_causal(inputs["mask"]):
        return _host_fallback(inputs)
    if not _device_available():
        import sys as _sys
        print("kernel: NeuronCores unreachable; computing on host",
              file=_sys.stderr)
        return _host_fallback(inputs)

    try:
        from concourse.bass_utils import run_bass_kernel_spmd

        nc = get_nc()
        in_maps = []
        for c in range(NCORES):
            b, hg = c // NG, c % NG
            in_maps.append(make_core_inputs(inputs, b, hg))
        res = run_bass_kernel_spmd(nc, in_maps, core_ids=list(range(NCORES)))
        out = np.zeros((B, S, D), dtype=np.float32)
        for c in range(NCORES):
            out[c // NG] += np.asarray(res.results[c]["y"], np.float32)
        return out
    except Exception as e:
        import sys as _sys
        import traceback

        traceback.print_exc()
        print(f"kernel: device path failed ({e!r}); computing on host",
              file=_sys.stderr)
        return _host_fallback(inputs)


# revision 39
# speedup vs baseline: 1.1890x; 1.1890x over previous
"""Trainium2 Bass kernel: attention layer with RoPE + gated adapter cross-attention.

Problem: B=2, S=2048, D=2048, H=16 heads (HD=128), adapter_len L=10.

  xq/xk/xv = x @ wq/wk/wv   (per-head reshape)
  xq, xk = rope(xq), rope(xk)
  out  = softmax(xq xk^T * scale + causal_mask) @ xv
  out += gate_h * softmax(xq ak^T * scale) @ av     (ak/av = adapter @ wk/wv)
  y    = out @ wo

Sharding (8 NeuronCores): 2 batch shards x 4 head-groups of 4 heads.
Each core computes attention for its (batch, 4 heads) and the partial
output projection with its 512 rows of wo; the host sums 4 partials per
batch element.  No on-device collectives.

Device layouts (per core):
  xT     [D, S]    bf16  x[b] transposed (feature-major)
  wq/wk  [D, 512]  bf16  column slice, RoPE-deinterleave column permutation
  wv     [D, 512]  bf16  column slice (no permutation)
  wo     [512, D]  bf16  row slice (no permutation)
  csq2   [128, S]  bf16  rows 0:64 cos.T*scale, 64:128 sin.T*scale
  csq2s  [128, S]  bf16  rows 0:64 sin.T*scale, 64:128 cos.T*scale
  csk2   [128, S]  bf16  [cos.T; sin.T]  (for k, unscaled)
  csk2s  [128, S]  bf16  [sin.T; cos.T]
  akT    [128, 40] bf16  adapter-K transposed per head (host-computed)
  avg    [10, 512] bf16  adapter-V, pre-scaled by per-head gate (host)
  y      [S, D]    f32   partial output (ExternalOutput)

The RoPE trick: permuting wq/wk columns so each head's features are
[even0..even63, odd0..odd63] makes the rotation act on partition halves.
With cos/sin packed as [c;s] and [s;c] 128-partition tiles, the rotation
is 4 bf16 DVE mults + a sub + an add, all [64,512] with matched base
partitions (a DVE two-SBUF-input constraint); the f32 PSUM projection is
downcast once on ACT.

Softmax: scores computed transposed ([k, q]); constant-shift exp(s - 8)
(softmax-invariant).  Denominators ride the PE as M=128 ones-matmuls
(one per PAIR of prob tiles, pre-added in place on DVE) accumulating
alongside PV; M=128 output arrives partition-broadcast, so 1/sum is a
single [128,512] reciprocal_approx_fast on DVE and no GpSimd step.
The inner loop is balanced: PE ~540ns/ki vs ACT exp ~547ns/ki --
shifting more denominator work to either engine measured slower.  The
adapter gate is folded into the host-computed adapter-V.

Schedule: fully streamed per 512-query J-block --
  QK-proj+RoPE(J) -> V-proj(J) -> attention(h=0..3, J) -> wo(J-1)
so the PE never waits on a phase boundary and the wo matmuls trail one
block behind the epilogues they consume.  Output y is bf16 (halves the
store bandwidth; host sums partials in f32).
"""

import numpy as np
import ml_dtypes

B, S, D, H, HD, L = 2, 2048, 2048, 16, 128, 10
NCORES = 8
NG = 4            # head-group shards
NH = H // NG      # heads per core
DH = NH * HD      # 512: per-core projection width
QT = 512          # query chunk (free dim of most matmuls)
NJ = S // QT      # 4
KT = 128          # key tile
DKT = 128         # contraction tile
NDK = D // DKT    # 16
NST = S // 128    # 16 s-tiles
SCALE = 1.0 / float(np.sqrt(HD))

_BF16 = ml_dtypes.bfloat16
_NC_CACHE = {}


def _build_nc():
    """Build + compile the per-core Bacc graph (same graph on all cores)."""
    from contextlib import ExitStack

    import concourse.tile as tile
    from concourse import bacc, mybir

    f32, bf16 = mybir.dt.float32, mybir.dt.bfloat16
    AF = mybir.ActivationFunctionType
    OP = mybir.AluOpType

    nc = bacc.Bacc("TRN2", target_bir_lowering=False, debug=False,
                   num_devices=NCORES)
    xT = nc.dram_tensor("xT", [D, S], bf16, kind="ExternalInput").ap()
    wq = nc.dram_tensor("wq", [D, DH], bf16, kind="ExternalInput").ap()
    wk = nc.dram_tensor("wk", [D, DH], bf16, kind="ExternalInput").ap()
    wv = nc.dram_tensor("wv", [D, DH], bf16, kind="ExternalInput").ap()
    wo = nc.dram_tensor("wo", [DH, D], bf16, kind="ExternalInput").ap()
    csq2 = nc.dram_tensor("csq2", [128, S], bf16, kind="ExternalInput").ap()
    csq2s = nc.dram_tensor("csq2s", [128, S], bf16, kind="ExternalInput").ap()
    csk2 = nc.dram_tensor("csk2", [128, S], bf16, kind="ExternalInput").ap()
    csk2s = nc.dram_tensor("csk2s", [128, S], bf16, kind="ExternalInput").ap()
    akT = nc.dram_tensor("akT", [128, NH * L], bf16, kind="ExternalInput").ap()
    avg = nc.dram_tensor("avg", [L, DH], bf16, kind="ExternalInput").ap()
    dmask = nc.dram_tensor("dmask", [128, 4 * QT], bf16,
                           kind="ExternalInput").ap()
    y = nc.dram_tensor("y", [S, D], bf16, kind="ExternalOutput").ap()

    with tile.TileContext(nc) as tc:
        with ExitStack() as ctx:
            pers = ctx.enter_context(tc.tile_pool(name="pers", bufs=1))
            ps = ctx.enter_context(
                tc.tile_pool(name="ps", space="PSUM", bufs=4))
            sb = ctx.enter_context(tc.tile_pool(name="sb", bufs=4))

            def ptile(shape, dt, nm):
                return pers.tile(shape, dt, name=nm, tag=nm)

            wq_t = [ptile([128, DH], bf16, f"twq{dk}") for dk in range(NDK)]
            wk_t = [ptile([128, DH], bf16, f"twk{dk}") for dk in range(NDK)]
            wv_t = [ptile([128, DH], bf16, f"twv{dk}") for dk in range(NDK)]
            wo_t = [ptile([128, D], bf16, f"two{f}") for f in range(NH)]
            csq2_t = ptile([128, S], bf16, "tcsq2")
            csq2s_t = ptile([128, S], bf16, "tcsq2s")
            csk2_t = ptile([128, S], bf16, "tcsk2")
            csk2s_t = ptile([128, S], bf16, "tcsk2s")
            akT_t = ptile([128, NH * L], bf16, "takT")
            avg_t = ptile([L, DH], bf16, "tavg")
            ones_t = ptile([128, 128], bf16, "tones")
            m8_t = ptile([128, 1], f32, "tm8")
            dm_t = ptile([128, 4 * QT], bf16, "tdm")
            kT_t = [ptile([128, S], bf16, f"tkT{h}") for h in range(NH)]
            v_t = [ptile([128, DH], bf16, f"tv{si}") for si in range(NST)]

            xt_all = {}

            def load_x(J):
                """Emit x DMAs for block J; J=0 interleaves wq/wk so the
                first accumulation group's operands land ASAP (in-order
                DMA queues: emission order = arrival order)."""
                jsl = slice(J * QT, (J + 1) * QT)
                xt = []
                for dk in range(NDK):
                    t = sb.tile([128, QT], bf16, tag="x", bufs=22,
                                name=f"x{J}_{dk}")
                    nc.sync.dma_start(
                        t[:], xT[dk * 128:(dk + 1) * 128, jsl])
                    xt.append(t)
                    if J == 0:
                        # block 0 computes V first: pair x with wv so the
                        # PE can start after ~2 tiles instead of 6MB
                        nc.sync.dma_start(
                            wv_t[dk][:], wv[dk * 128:(dk + 1) * 128, :])
                xt_all[J] = xt

            # ---- initial DMA emission (consumption order)
            load_x(0)
            for dk in range(NDK):
                nc.sync.dma_start(wq_t[dk][:], wq[dk * 128:(dk + 1) * 128, :])
                nc.sync.dma_start(wk_t[dk][:], wk[dk * 128:(dk + 1) * 128, :])
            for t, src in ((csq2_t, csq2), (csq2s_t, csq2s),
                           (csk2_t, csk2), (csk2s_t, csk2s)):
                nc.sync.dma_start(t[:], src[:, :])
            nc.sync.dma_start(dm_t[:], dmask[:, :])
            nc.sync.dma_start(akT_t[:], akT[:, :])
            nc.sync.dma_start(avg_t[:], avg[:, :])
            nc.gpsimd.memset(ones_t[:], 1.0)
            nc.gpsimd.memset(m8_t[:], -8.0)
            load_x(1)
            for f in range(NH):
                nc.sync.dma_start(wo_t[f][:], wo[f * 128:(f + 1) * 128, :])

            ao_all = {}   # (h, J) -> [128, QT] bf16 unnormalized-combined

            def emit_wo(J):
                """Output projection for s-tiles of block J (trails one
                block behind attention so epilogues are long done)."""
                fine = (J == NJ - 1)  # last block: small stores spread the
                for sv in range(4):   # tail across more DMA queues
                    si = 4 * J + sv
                    ssl = slice(si * 128, (si + 1) * 128)
                    lsl = slice(sv * 128, (sv + 1) * 128)
                    for half in range(2):
                        ysb = sb.tile([128, D // 2], bf16, tag="y", bufs=3,
                                      name=f"y{si}_{half}")
                        for n2 in range(2):
                            n = 2 * half + n2
                            nsl = slice(n * QT, (n + 1) * QT)
                            yp = ps.tile([128, QT], f32, tag="s", bufs=4,
                                         name=f"yp{si}_{n}")
                            for f in range(NH):
                                nc.tensor.matmul(
                                    yp[:], ao_all[(f, J)][:, lsl],
                                    wo_t[f][:, nsl],
                                    start=(f == 0), stop=(f == NH - 1))
                            nc.scalar.copy(
                                ysb[:, n2 * QT:(n2 + 1) * QT], yp[:])
                            if fine:
                                nc.sync.dma_start(
                                    y[ssl, nsl],
                                    ysb[:, n2 * QT:(n2 + 1) * QT])
                        if not fine:
                            nc.sync.dma_start(
                                y[ssl, half * (D // 2):(half + 1) * (D // 2)],
                                ysb[:])

            for J in range(NJ):
                if 1 <= J and J + 1 < NJ:
                    load_x(J + 1)
                jsl = slice(J * QT, (J + 1) * QT)
                xt = xt_all[J]

                def emit_v(Jv):
                    xv = xt_all[Jv]
                    for sv in range(4):
                        si = 4 * Jv + sv
                        vp = ps.tile([128, DH], f32, tag="s", bufs=4,
                                     name=f"vp{si}")
                        for dk in range(NDK):
                            nc.tensor.matmul(
                                vp[:], xv[dk][:, sv * 128:(sv + 1) * 128],
                                wv_t[dk][:], start=(dk == 0),
                                stop=(dk == NDK - 1))
                        nc.scalar.copy(v_t[si][:], vp[:])

                # block 0 races the initial DMA: V-proj needs only x+wv
                # (4MB), so run it first while wq/wk are still streaming
                if J == 0:
                    emit_v(0)

                # ---- QK projections + RoPE for this block, per head
                qT_J = {}
                for h in range(NH):
                    hsl = slice(h * 128, (h + 1) * 128)
                    for w_t, cs2, cs2s, pfx in (
                            (wq_t, csq2_t, csq2s_t, "q"),
                            (wk_t, csk2_t, csk2s_t, "k")):
                        pp = ps.tile([128, QT], f32, tag="s", bufs=4,
                                     name=f"pp{pfx}{J}_{h}")
                        for dk in range(NDK):
                            nc.tensor.matmul(
                                pp[:], w_t[dk][:, hsl], xt[dk][:],
                                start=(dk == 0), stop=(dk == NDK - 1))
                        psb = sb.tile([128, QT], bf16, tag="psb", bufs=4,
                                      name=f"pb{pfx}{J}_{h}")
                        nc.scalar.copy(psb[:], pp[:])
                        if pfx == "q":
                            dst = sb.tile([128, QT], bf16, tag="qT", bufs=6,
                                          name=f"qT{J}_{h}")
                            qT_J[h] = dst
                            d_top, d_bot = dst[0:64, :], dst[64:128, :]
                        else:
                            d_top = kT_t[h][0:64, jsl]
                            d_bot = kT_t[h][64:128, jsl]
                        # all-bf16 products; DVE two-SBUF-input ops need
                        # matching base partitions, so pair 0:64 with 0:64
                        # and 64:128 with 64:128 (cs2=[c;s], cs2s=[s;c])
                        tt = []
                        for i, (vsl, cst) in enumerate((
                                (slice(0, 64), cs2), (slice(64, 128), cs2),
                                (slice(0, 64), cs2s),
                                (slice(64, 128), cs2s))):
                            t = sb.tile([64, QT], bf16, tag="rt", bufs=8,
                                        name=f"t{i}{pfx}{J}_{h}")
                            nc.vector.tensor_tensor(
                                t[:], psb[vsl, :], cst[vsl, jsl], op=OP.mult)
                            tt.append(t)
                        nc.vector.tensor_tensor(
                            d_top, tt[0][:], tt[1][:], op=OP.subtract)
                        nc.vector.tensor_tensor(
                            d_bot, tt[2][:], tt[3][:], op=OP.add)

                # ---- V projection for this block's 4 s-tiles
                if J > 0:
                    emit_v(J)

                # ---- attention for each head at this block
                for h in range(NH):
                    hsl = slice(h * 128, (h + 1) * 128)
                    qs = qT_J[h][:, :]
                    nki = 4 * J + 4
                    ops_ = ps.tile([128, QT], f32, tag="o", bufs=2,
                                   name=f"o{h}_{J}")
                    sums = ps.tile([128, QT], f32, tag="sum", bufs=2,
                                   name=f"sm{h}_{J}")
                    # adapter scores issue before the ki loop; their exp
                    # latency hides behind the main attention matmuls
                    ap_ = ps.tile([L, QT], f32, tag="s", bufs=4,
                                  name=f"ap{h}_{J}")
                    nc.tensor.matmul(ap_[:], akT_t[:, h * L:(h + 1) * L],
                                     qs, start=True, stop=True)
                    pa = sb.tile([L, QT], bf16, tag="pa", bufs=2,
                                 name=f"pa{h}_{J}")
                    nc.scalar.activation(pa[:], ap_[:], AF.Exp,
                                         bias=m8_t[0:L, :])
                    # software-pipelined ki loop (depth 2): the scores
                    # matmul of ki+1..ki+2 issues on PE before the PV of
                    # ki, hiding the PE->ACT(exp)->PE round trip.
                    #
                    # Denominator: M=128 ones matmuls (the result arrives
                    # already partition-broadcast, so no GpSimd step), one
                    # per PAIR of prob tiles pre-added on the idle DVE.
                    # This halves the sums PE cost and balances the inner
                    # loop: PE ~540ns/ki vs ACT exp ~547ns/ki (dropping
                    # sums entirely makes attention ACT-bound and stalls
                    # the in-order PE queue -- measured slower).
                    pend = []
                    half = []
                    n_sums = (nki + 1) // 2
                    sums_emitted = 0

                    def emit_sums(a, b):
                        nonlocal sums_emitted
                        ka, ta, qa = a
                        if b is not None:
                            kb, tb, qb = b
                            # in-place pair add on b's visible range; the
                            # [qa:qb) region keeps a's values alone
                            nc.vector.tensor_tensor(
                                ta[:, qb:], ta[:, qb:], tb[:, qb:],
                                op=OP.add)
                        nc.tensor.matmul(
                            sums[:, qa:], ones_t[:, :], ta[:, qa:],
                            start=(sums_emitted == 0),
                            stop=(sums_emitted == n_sums - 1),
                            skip_group_check=True)
                        sums_emitted += 1

                    def flush_one():
                        ki, pt_t, q0 = pend.pop(0)
                        nc.tensor.matmul(
                            ops_[:, q0:], v_t[ki][:, hsl], pt_t[:, q0:],
                            start=(ki == 0), stop=(ki == nki - 1),
                            skip_group_check=True)
                        half.append((ki, pt_t, q0))
                        if len(half) == 2:
                            emit_sums(half[0], half[1])
                            half.clear()

                    for ki in range(nki):
                        di = ki - 4 * J
                        q0 = di * 128 if di >= 0 else 0
                        sp = ps.tile([128, QT], f32, tag="s", bufs=4,
                                     name=f"sp{h}_{J}_{ki}")
                        nc.tensor.matmul(
                            sp[:, q0:], kT_t[h][:, ki * KT:(ki + 1) * KT],
                            qs[:, q0:], start=True, stop=True)
                        pt = sb.tile([128, QT], bf16, tag="pt", bufs=6,
                                     name=f"pt{h}_{J}_{ki}")
                        # exp(s - 8): softmax-invariant shift guards
                        # f32 overflow for any plausible score scale
                        nc.scalar.activation(pt[:, q0:], sp[:, q0:],
                                             AF.Exp, bias=m8_t[:, :])
                        if di >= 0:
                            ptm = sb.tile([128, QT], bf16, tag="ptm",
                                          bufs=4, name=f"pm{h}_{J}_{ki}")
                            nc.vector.tensor_tensor(
                                ptm[:, q0:], pt[:, q0:],
                                dm_t[:, di * QT + q0:(di + 1) * QT],
                                op=OP.mult)
                            pt = ptm
                        pend.append((ki, pt, q0))
                        if len(pend) > 2:
                            flush_one()
                    while pend:
                        flush_one()
                    if half:
                        emit_sums(half[0], None)
                        half.clear()
                    # adapter PV + sums (pa ready by now)
                    asums = ps.tile([128, QT], f32, tag="sum", bufs=2,
                                    name=f"as{h}_{J}")
                    nc.tensor.matmul(asums[:], ones_t[0:L, :], pa[:],
                                     start=True, stop=True)
                    avp_ = ps.tile([128, QT], f32, tag="s", bufs=4,
                                   name=f"av{h}_{J}")
                    nc.tensor.matmul(avp_[:], avg_t[:, hsl], pa[:],
                                     start=True, stop=True)
                    # epilogue: 1/sum via the fast custom-DVE reciprocal
                    # (~5x the plain DVE reciprocal; sums are positive and
                    # well inside range; DVE cost scales with free size
                    # only, so the [128,QT] broadcast shape is free).  ACT
                    # Ln/Exp would be fast per-op but each function switch
                    # costs a ~1.3us table reload.
                    # adapter term first: frees avp_'s "s" PSUM buf ASAP
                    # (it otherwise gates the next block's proj allocs)
                    rab = sb.tile([128, QT], f32, tag="ep2", bufs=2,
                                  name=f"rc{h}_{J}")
                    nc.vector.reciprocal_approx_fast(out=rab[:],
                                                     in_=asums[:])
                    t_a = sb.tile([128, QT], f32, tag="ep3", bufs=2,
                                  name=f"ta{h}_{J}")
                    nc.vector.tensor_tensor(t_a[:], avp_[:], rab[:],
                                            op=OP.mult)
                    rb = sb.tile([128, QT], f32, tag="ep2", bufs=2,
                                 name=f"rb{h}_{J}")
                    nc.vector.reciprocal_approx_fast(out=rb[:], in_=sums[:])
                    t_o = sb.tile([128, QT], f32, tag="ep3", bufs=2,
                                  name=f"to{h}_{J}")
                    nc.vector.tensor_tensor(t_o[:], ops_[:], rb[:],
                                            op=OP.mult)
                    ao = sb.tile([128, QT], bf16, tag="ao", bufs=8,
                                 name=f"ao{h}_{J}")
                    ao_all[(h, J)] = ao
                    nc.vector.tensor_tensor(ao[:], t_o[:], t_a[:], op=OP.add)

                # ---- output projection trails one block behind
                if J >= 1:
                    emit_wo(J - 1)
            emit_wo(NJ - 1)
    nc.compile()
    return nc


def get_nc():
    if "nc" not in _NC_CACHE:
        _NC_CACHE["nc"] = _build_nc()
    return _NC_CACHE["nc"]


# ---------------------------------------------------------------- host side

def _rope_perm():
    """Column permutation making each head's features [evens..., odds...]."""
    blk = np.concatenate([np.arange(0, 128, 2), np.arange(1, 128, 2)])
    return np.concatenate([h * 128 + blk for h in range(NH)])


def _diag_masks():
    """dmask[k_local, di*QT + q_local] = 1 if (di*128 + k_local) <= q_local."""
    out = np.zeros((128, 4 * QT), dtype=np.float32)
    kl = np.arange(128)[:, None]
    ql = np.arange(QT)[None, :]
    for di in range(4):
        out[:, di * QT:(di + 1) * QT] = (di * 128 + kl) <= ql
    return out


def make_core_inputs(inputs, b, hg):
    """Build the in_map for core (b, hg). All arrays C-contiguous."""
    x = np.asarray(inputs["x"], dtype=np.float32)
    wq = np.asarray(inputs["wq"], dtype=np.float32)
    wk = np.asarray(inputs["wk"], dtype=np.float32)
    wv = np.asarray(inputs["wv"], dtype=np.float32)
    wo = np.asarray(inputs["wo"], dtype=np.float32)
    adapter = np.asarray(inputs["adapter"], dtype=np.float32)
    gate = np.asarray(inputs["gate"], dtype=np.float32)
    cos = np.asarray(inputs["freqs_cos"], dtype=np.float32)
    sin = np.asarray(inputs["freqs_sin"], dtype=np.float32)

    cols = slice(hg * DH, (hg + 1) * DH)
    perm = _rope_perm()
    bf = _BF16
    wkp = np.ascontiguousarray(wk[:, cols][:, perm])
    wvs = np.ascontiguousarray(wv[:, cols])
    # adapter K/V on host (tiny): ak in the same permuted feature basis as
    # q; av pre-scaled by the per-head gate so no gate multiply on device
    ak = adapter[0] @ wkp                      # [L, DH] permuted features
    av = adapter[0] @ wvs                      # [L, DH]
    gv = gate[0, hg * NH:(hg + 1) * NH, 0, 0]  # [NH]
    avg_ = (av.reshape(L, NH, HD) * gv[None, :, None]).reshape(L, DH)
    akT_ = np.concatenate(
        [ak[:, h * HD:(h + 1) * HD].T for h in range(NH)], axis=1)  # [128,40]
    cT, sT = cos.T, sin.T                      # [64, S]
    m = {
        "xT": np.ascontiguousarray(x[b].T).astype(bf),
        "wq": np.ascontiguousarray(wq[:, cols][:, perm]).astype(bf),
        "wk": wkp.astype(bf),
        "wv": wvs.astype(bf),
        "wo": np.ascontiguousarray(wo[cols, :]).astype(bf),
        "csq2": np.ascontiguousarray(
            np.concatenate([cT * SCALE, sT * SCALE], axis=0)).astype(bf),
        "csq2s": np.ascontiguousarray(
            np.concatenate([sT * SCALE, cT * SCALE], axis=0)).astype(bf),
        "csk2": np.ascontiguousarray(
            np.concatenate([cT, sT], axis=0)).astype(bf),
        "csk2s": np.ascontiguousarray(
            np.concatenate([sT, cT], axis=0)).astype(bf),
        "akT": np.ascontiguousarray(akT_).astype(bf),
        "avg": np.ascontiguousarray(avg_).astype(bf),
        "dmask": _diag_masks().astype(bf),
    }
    return m


def _mask_is_causal(mask):
    """True when mask[0,0] is the standard additive causal mask."""
    mk = np.asarray(mask)[0, 0]
    iu = np.triu_indices(S, k=1)
    il = np.tril_indices(S, k=0)
    return bool(np.all(mk[il] == 0.0) and np.all(mk[iu] < -1e8))


def _host_fallback(inputs):
    """Pure-numpy reference (used only if the mask is not causal)."""
    x = np.asarray(inputs["x"], dtype=np.float32)
    wq = np.asarray(inputs["wq"], dtype=np.float32)
    wk = np.asarray(inputs["wk"], dtype=np.float32)
    wv = np.asarray(inputs["wv"], dtype=np.float32)
    wo = np.asarray(inputs["wo"], dtype=np.float32)
    adapter = np.asarray(inputs["adapter"], dtype=np.float32)
    gate = np.asarray(inputs["gate"], dtype=np.float32)
    cos = np.asarray(inputs["freqs_cos"], dtype=np.float32)
    sin = np.asarray(inputs["freqs_sin"], dtype=np.float32)
    mask = np.asarray(inputs["mask"], dtype=np.float32)

    def rope(v):
        vv = v.reshape(*v.shape[:-1], HD // 2, 2)
        v0, v1 = vv[..., 0], vv[..., 1]
        c = cos[None, :, None, :]
        s = sin[None, :, None, :]
        out = np.stack([v0 * c - v1 * s, v0 * s + v1 * c], axis=-1)
        return out.reshape(v.shape)

    xq = rope((x @ wq).reshape(B, S, H, HD))
    xk = rope((x @ wk).reshape(B, S, H, HD))
    xv = (x @ wv).reshape(B, S, H, HD)
    scores = np.einsum("bqhd,bkhd->bhqk", xq, xk) * SCALE + mask
    scores -= scores.max(axis=-1, keepdims=True)
    p = np.exp(scores)
    p /= p.sum(axis=-1, keepdims=True)
    out = np.einsum("bhqk,bkhd->bqhd", p, xv)
    ak = (adapter[0] @ wk).reshape(L, H, HD)
    av = (adapter[0] @ wv).reshape(L, H, HD)
    asc = np.einsum("bqhd,khd->bhqk", xq, ak) * SCALE
    asc -= asc.max(axis=-1, keepdims=True)
    pa = np.exp(asc)
    pa /= pa.sum(axis=-1, keepdims=True)
    pa = gate * pa
    out = out + np.einsum("bhqk,khd->bqhd", pa, av)
    return (out.reshape(B, S, D) @ wo).astype(np.float32)


def _device_available():
    """Check the axon tunnel is reachable without claiming a device (a jax
    probe subprocess would grab a terminal session and could contend with
    the real run).  When no tunnel env is present, assume native devices."""
    import os
    import socket

    if not os.environ.get("TRN_TERMINAL_POOL_IPS"):
        import glob

        return bool(glob.glob("/dev/neuron*"))  # native path
    for port in (8082, 8083, 8087):
        s = socket.socket()
        s.settimeout(5)
        try:
            s.connect(("127.0.0.1", port))
            return True
        except OSError:
            continue
        finally:
            s.close()
    return False


def kernel(**inputs) -> np.ndarray:
    if not _mask_is_causal(inputs["mask"]):
        return _host_fallback(inputs)
    if not _device_available():
        import sys as _sys
        print("kernel: NeuronCores unreachable; computing on host",
              file=_sys.stderr)
        return _host_fallback(inputs)

    try:
        from concourse.bass_utils import run_bass_kernel_spmd

        nc = get_nc()
        in_maps = []
        for c in range(NCORES):
            b, hg = c // NG, c % NG
            in_maps.append(make_core_inputs(inputs, b, hg))
        res = run_bass_kernel_spmd(nc, in_maps, core_ids=list(range(NCORES)))
        out = np.zeros((B, S, D), dtype=np.float32)
        for c in range(NCORES):
            out[c // NG] += np.asarray(res.results[c]["y"], np.float32)
        return out
    except Exception as e:
        import sys as _sys
        import traceback

        traceback.print_exc()
        print(f"kernel: device path failed ({e!r}); computing on host",
              file=_sys.stderr)
        return _host_fallback(inputs)
